# revision 29
# baseline (speedup 1.0000x reference)
"""Trainium2 Bass kernel for a 2-layer GCN + global mean pool + MLP head.

Strategy (8 NeuronCores, SPMD):
  - Nodes (and their incident edges, grouped by destination) are sharded
    across the 8 cores; each core owns N/8 destination nodes.
  - Layer 1's edge gather is done ON THE HOST (x is an input): each core
    receives a pre-expanded [128, slots, 16] bf16 stream of
    x[src] * dinv[src] * dinv[dst] values (self-loops included), so the
    device does zero gather work for layer 1.  Aggregation is a one-hot
    matmul per 128-edge tile directly into a transposed [16, wg*128] PSUM
    batched over the whole window group.
  - Layer 2 gathers rows of the AllGather'ed (h1 @ W2) * dinv table with
    gpsimd dma_gather (dst-sorted edges, lo/hi split for int16 indices).
    The AllGather is chunked so it overlaps the tail of layer 1.
    One-hot values carry dinv[dst] (bf16), so the aggregation PSUM needs
    no per-window scale and SELU batches across window pairs.
    Self-loops are a diag(dinv[dst]) stationary against the local shard.
  - SELU is computed as m + alpha*(exp(min(z,0))-1) via Relu/Exp on the
    scalar engine; m and u feed the same accumulating matmul so no add is
    needed, and lambda is folded into the next weights host-side.
  - Mean-pool partial sums use one-hot-matmul (node -> graph id),
    AllReduce-summed; the tiny MLP head + log_softmax run redundantly.
"""

import os
import numpy as np
import ml_dtypes

import concourse.bacc as bacc
import concourse.bass as bass
import concourse.mybir as mybir
import concourse.tile as tile
from concourse.bass_utils import run_bass_kernel_spmd

F32 = mybir.dt.float32
F8 = mybir.dt.float8e4
BF16 = mybir.dt.bfloat16
I16 = mybir.dt.int16
AF = mybir.ActivationFunctionType
OP = mybir.AluOpType
NPBF16 = ml_dtypes.bfloat16
NPF8 = ml_dtypes.float8_e4m3

SELU_LAM = 1.0507009873554805
SELU_ALPHA = 1.6732632423543772

P = 128
NCORES = 8
AG_CHUNKED = False
LO_SHARDS = 5  # table split for int16 gather indices: lo = first 5 shards


def _groups(W, grp):
    out = []
    w = 0
    while w < W:
        wg = min(grp, W - w)
        out.append((w, wg))
        w += wg
    return out


class Lay1:
    """Layer-1 host-expanded layout: window-major dense slots per group."""

    def __init__(self, n_nodes, cnt1_cw, grp):
        self.NSH = n_nodes // NCORES
        self.W = -(-self.NSH // P)
        self.groups = _groups(self.W, grp)
        self.T = []
        self.base = []
        b = 0
        for (w0, wg) in self.groups:
            t = max(
                -(-int(cnt1_cw[c, w]) // P)
                for c in range(NCORES)
                for w in range(w0, w0 + wg)
            )
            self.T.append(t)
            self.base.append(b)
            b += wg * t
        self.S_TOT = b


class Lay2:
    """Layer-2 gather layout: group-contiguous lo/hi int16 index streams.

    Edges of a window group are packed back-to-back (window-major) into one
    lo and one hi stream per group; only the stream tail is padded.  Each
    window's edges then span a *static* tile range (min/max over cores of
    its per-core prefix offsets); boundary tiles shared by two windows get
    one matmul (with a window-masked one-hot block) per window."""

    def __init__(self, n_nodes, nlo_cw, nhi_cw, grp):
        self.NSH = n_nodes // NCORES
        self.W = -(-self.NSH // P)
        self.groups = _groups(self.W, grp)
        self.T_LO, self.T_HI = [], []
        self.rng_lo, self.rng_hi = [], []   # per group: list per window (t0, t1)
        self.colbase = []                    # per group: per window, per stream col base
        self.lo_col, self.hi_col = [], []    # idx slab col offsets
        self.slot_base = []                  # gt2 slot base per group (0; per-group tiles)
        col = 0
        ohcol = 0
        for g, (w0, wg) in enumerate(self.groups):
            lo_off = np.zeros((NCORES, wg + 1), np.int64)
            hi_off = np.zeros((NCORES, wg + 1), np.int64)
            for c in range(NCORES):
                lo_off[c, 1:] = np.cumsum(nlo_cw[c, w0 : w0 + wg])
                hi_off[c, 1:] = np.cumsum(nhi_cw[c, w0 : w0 + wg])
            tl = int(max(-(-lo_off[c, wg] // P) for c in range(NCORES)))
            th = int(max(-(-hi_off[c, wg] // P) for c in range(NCORES)))
            self.T_LO.append(tl)
            self.T_HI.append(th)
            rl, rh, cb = [], [], []
            for k in range(wg):
                lt0 = int(min(lo_off[c, k] // P for c in range(NCORES)))
                lt1 = int(max(-(-lo_off[c, k + 1] // P) for c in range(NCORES)))
                ht0 = int(min(hi_off[c, k] // P for c in range(NCORES)))
                ht1 = int(max(-(-hi_off[c, k + 1] // P) for c in range(NCORES)))
                rl.append((lt0, lt1))
                rh.append((ht0, ht1))
                cb.append(ohcol)
                ohcol += (lt1 - lt0 + ht1 - ht0) * P
            self.rng_lo.append(rl)
            self.rng_hi.append(rh)
            self.colbase.append(cb)
            self.lo_col.append(col)
            col += tl * 8
            self.hi_col.append(col)
            col += th * 8
        self.IDX_COLS = col
        self.OH_COLS = ohcol
        self.lo_off_all = None  # filled by host_prep


def edge_partition(inputs, n_nodes, split):
    """Sort edges by destination; per-(core,window) counts (no self-loops)."""
    ei = np.asarray(inputs["edge_index"], np.int64)
    src, dst = ei[0], ei[1]
    order = np.argsort(dst, kind="stable")
    s, d = src[order], dst[order]
    nsh = n_nodes // NCORES
    W = -(-nsh // P)
    bounds = [c * nsh + w * P for c in range(NCORES) for w in range(W)] + [n_nodes]
    cut = np.searchsorted(d, np.asarray(bounds))
    nlo = np.zeros((NCORES, W), np.int64)
    nhi = np.zeros((NCORES, W), np.int64)
    cnt1 = np.zeros((NCORES, W), np.int64)
    for i in range(NCORES * W):
        sw = s[cut[i] : cut[i + 1]]
        c, w = i // W, i % W
        rows = min(P, nsh - w * P)
        l = int((sw < split).sum())
        nlo[c, w] = l
        nhi[c, w] = len(sw) - l
        cnt1[c, w] = len(sw) + rows  # + self-loops
    return s, d, cut, nlo, nhi, cnt1


def host_prep(inputs, s, d, cut, l1, l2, n_nodes, n_graphs, split):
    N, G = n_nodes, n_graphs
    W = l1.W
    NSH = l1.NSH
    x = np.asarray(inputs["x"], np.float32)
    batch = np.asarray(inputs["batch"], np.int64)
    D_IN = x.shape[1]

    deg = np.bincount(d, minlength=N).astype(np.float64) + 1.0  # + self loop
    dinv = (1.0 / np.sqrt(deg)).astype(np.float32)
    xs = (x * dinv[:, None]).astype(np.float32)

    cnt = np.bincount(batch, minlength=G).astype(np.float32)
    cntinv = (SELU_LAM / np.maximum(cnt, 1.0)).astype(np.float32)  # λ2 folded

    per_core = []
    for c in range(NCORES):
        # ---------- layer 1: host-expanded values + one-hots ----------
        gx1 = np.zeros((l1.S_TOT * P, 16), np.float32)
        dl1 = np.full((P, l1.S_TOT), -1.0, np.float32)
        for g, (w0, wg) in enumerate(l1.groups):
            T = l1.T[g]
            for k in range(wg):
                w = w0 + k
                i = c * W + w
                sw = s[cut[i] : cut[i + 1]]
                dw = d[cut[i] : cut[i + 1]] - (c * NSH + w * P)
                rows = min(P, NSH - w * P)
                base = c * NSH + w * P
                srcs = np.concatenate([sw, np.arange(base, base + rows)])
                dsts = np.concatenate([dw, np.arange(rows)]).astype(np.int64)
                ddst = dinv[c * NSH + w * P + dsts]
                n_e = len(srcs)
                slot0 = l1.base[g] + k * T
                gx1[slot0 * P : slot0 * P + n_e, :D_IN] = xs[srcs, :D_IN] * ddst[:, None]
                flat = np.full(T * P, -1.0, np.float32)
                flat[:n_e] = dsts
                dl1[:, slot0 : slot0 + T] = flat.reshape(T, P).T
        oh1 = (dl1[:, :, None] == np.arange(P, dtype=np.float32)[None, None, :])
        oh1 = oh1.astype(NPF8).reshape(P, l1.S_TOT * P)
        gx1v = gx1.reshape(l1.S_TOT, P, 16).transpose(1, 0, 2).reshape(P, l1.S_TOT * 16)
        gx1v = gx1v.astype(NPBF16)

        # per-window dst dinv / graph one-hot
        dinv_w = np.zeros((P, W), np.float32)
        batchloc = np.full((P, W), -1.0, np.float32)
        base = c * NSH
        for w in range(W):
            rows = min(P, NSH - w * P)
            dinv_w[:rows, w] = dinv[base + w * P : base + w * P + rows]
            batchloc[:rows, w] = batch[base + w * P : base + w * P + rows].astype(np.float32)
        ohg = (batchloc[:, :, None] == np.arange(G, dtype=np.float32)[None, None, :])
        ohg = ohg.astype(NPBF16).reshape(P, W * G)

        # ---------- layer 2: group-contiguous idx streams + one-hots ----------
        idx_slab = np.zeros((16, l2.IDX_COLS), np.int16)
        oh2 = np.zeros((P, l2.OH_COLS), np.float32)
        for g, (w0, wg) in enumerate(l2.groups):
            lo_lists, hi_lists, lo_d, hi_d = [], [], [], []
            for k in range(wg):
                w = w0 + k
                i = c * W + w
                sw = s[cut[i] : cut[i + 1]]
                dw = (d[cut[i] : cut[i + 1]] - (c * NSH + w * P)).astype(np.int64)
                m = sw < split
                lo_lists.append(sw[m]); lo_d.append(dw[m])
                hi_lists.append(sw[~m] - split); hi_d.append(dw[~m])
            for lists, dls, rng, tl, col0 in (
                (lo_lists, lo_d, l2.rng_lo[g], l2.T_LO[g], l2.lo_col[g]),
                (hi_lists, hi_d, l2.rng_hi[g], l2.T_HI[g], l2.hi_col[g]),
            ):
                flat = np.concatenate(lists) if lists else np.zeros(0, np.int64)
                st = np.zeros(tl * P, np.int16)
                st[: len(flat)] = flat.astype(np.int16)
                idx_slab[:, col0 : col0 + tl * 8] = st.reshape(-1, 16).T
            # one-hot blocks per (window, stream, tile)
            lo_off = np.concatenate([[0], np.cumsum([len(x) for x in lo_lists])])
            hi_off = np.concatenate([[0], np.cumsum([len(x) for x in hi_lists])])
            lo_flat_d = np.concatenate(lo_d) if lo_d else np.zeros(0, np.int64)
            hi_flat_d = np.concatenate(hi_d) if hi_d else np.zeros(0, np.int64)
            for k in range(wg):
                w = w0 + k
                colp = l2.colbase[g][k]
                for (t0, t1), off, fd in (
                    (l2.rng_lo[g][k], lo_off, lo_flat_d),
                    (l2.rng_hi[g][k], hi_off, hi_flat_d),
                ):
                    for t in range(t0, t1):
                        p0, p1 = t * P, (t + 1) * P
                        a = max(p0, int(off[k])); b = min(p1, int(off[k + 1]))
                        if b > a:
                            rows = np.arange(a - p0, b - p0)
                            dl = fd[a:b]
                            oh2[rows, colp + dl] = dinv_w[dl, w]
                        colp += P
        oh2 = oh2.astype(NPBF16)

        # self-loop stationary: diag(dinv[d]) per window, bf16
        selfd = np.zeros((P, W * P), NPBF16)
        for w in range(W):
            selfd[:, w * P : (w + 1) * P][np.arange(P), np.arange(P)] = dinv_w[:, w].astype(NPBF16)

        per_core.append({
            "gx1": gx1v,
            "oht1": oh1,
            "idxs": np.tile(idx_slab, (8, 1)),
            "oht2": oh2,
            "selfd": selfd,
            "ohgt": ohg,
            "dinv_w": dinv_w,
        })

    # ---------- shared constants (SELU lambdas folded downstream) ----------
    D_HID = np.asarray(inputs["W1"]).shape[1]
    W1p = np.zeros((16, D_HID), NPBF16)
    W1p[:D_IN] = np.asarray(inputs["W1"], np.float32).astype(NPBF16)
    W2 = np.asarray(inputs["W2"], np.float32) * SELU_LAM  # λ1
    W2_sb = np.concatenate([W2[:P, :], W2[P:, :]], axis=1).astype(NPBF16)
    b1 = np.asarray(inputs["b1"], np.float32).reshape(2, P).T.copy()
    b2b = np.tile(np.asarray(inputs["b2"], np.float32)[None, :], (P, 1))
    fc1 = np.asarray(inputs["fc1_w"], np.float32) * SELU_LAM  # λ3
    fc1_sb = np.concatenate([fc1[:P, :], fc1[P:, :]], axis=1).astype(NPBF16)
    fc1b = np.asarray(inputs["fc1_b"], np.float32).reshape(P, 1)
    fc2 = (np.asarray(inputs["fc2_w"], np.float32) * SELU_LAM).astype(NPBF16)  # λ4
    N_CLS = fc2.shape[1]
    fc2b = np.zeros((P, 1), np.float32)
    fc2b[:N_CLS, 0] = np.asarray(inputs["fc2_b"], np.float32)
    ident = np.eye(P, dtype=np.float32)
    cntinv2 = np.tile(cntinv[None, :], (P, 2))

    shared = {
        "W1p": W1p,
        "W2_sb": W2_sb,
        "b1h": b1,
        "nb1h": -b1,
        "b2b": b2b,
        "fc1_sb": fc1_sb,
        "fc1b": fc1b,
        "nfc1b": -fc1b,
        "fc2_sb": fc2,
        "fc2b": fc2b,
        "ident": ident,
        "cntinv2": cntinv2,
        "has_b1": bool(np.any(b1)),
        "has_b2": bool(np.any(b2b)),
    }
    for im in per_core:
        for k, v in shared.items():
            if not k.startswith("has_"):
                im[k] = v
    return per_core, shared


def build_nc(l1, l2, n_nodes, n_graphs, d_hid, d_fc, n_cls, has_b1, has_b2):
    nc = bacc.Bacc("TRN2", target_bir_lowering=False, debug=False,
                   num_devices=NCORES, num_swdge_queues=2)
    N, G, W = n_nodes, n_graphs, l1.W
    NSH = l1.NSH
    DH = d_hid
    SH2 = W * P
    SPLIT = LO_SHARDS * NSH

    gx1_d = nc.dram_tensor("gx1", [P, l1.S_TOT * 16], BF16, kind="ExternalInput")
    oht1_d = nc.dram_tensor("oht1", [P, l1.S_TOT * P], F8, kind="ExternalInput")
    idxs = nc.dram_tensor("idxs", [P, l2.IDX_COLS], I16, kind="ExternalInput")
    oht2_d = nc.dram_tensor("oht2", [P, l2.OH_COLS], BF16, kind="ExternalInput")
    selfd_d = nc.dram_tensor("selfd", [P, W * P], BF16, kind="ExternalInput")
    dinv_d = nc.dram_tensor("dinv_w", [P, W], F32, kind="ExternalInput")
    ohgt_d = nc.dram_tensor("ohgt", [P, W * G], BF16, kind="ExternalInput")
    W1p_d = nc.dram_tensor("W1p", [16, DH], BF16, kind="ExternalInput")
    W2_d = nc.dram_tensor("W2_sb", [P, 2 * DH], BF16, kind="ExternalInput")
    b1_d = nc.dram_tensor("b1h", [P, 2], F32, kind="ExternalInput")
    nb1_d = nc.dram_tensor("nb1h", [P, 2], F32, kind="ExternalInput")
    b2b_d = nc.dram_tensor("b2b", [P, DH], F32, kind="ExternalInput")
    fc1_d = nc.dram_tensor("fc1_sb", [P, 2 * d_fc], BF16, kind="ExternalInput")
    fc1b_d = nc.dram_tensor("fc1b", [P, 1], F32, kind="ExternalInput")
    nfc1b_d = nc.dram_tensor("nfc1b", [P, 1], F32, kind="ExternalInput")
    fc2_d = nc.dram_tensor("fc2_sb", [d_fc, n_cls], BF16, kind="ExternalInput")
    fc2b_d = nc.dram_tensor("fc2b", [P, 1], F32, kind="ExternalInput")
    ident_d = nc.dram_tensor("ident", [P, P], F32, kind="ExternalInput")
    cntinv2_d = nc.dram_tensor("cntinv2", [P, 2 * G], F32, kind="ExternalInput")

    out_d = nc.dram_tensor("out", [G, n_cls], F32, kind="ExternalOutput")

    shard2 = nc.dram_tensor("shard2", [SH2, DH], F8)
    h2t = nc.dram_tensor("h2t", [NCORES, NSH, DH], F8, addr_space="Shared")
    pool_part = nc.dram_tensor("pool_part", [2 * P, G], F32)
    pool_sum = nc.dram_tensor("pool_sum", [2 * P, G], F32, addr_space="Shared")

    # AllGather chunks: window-aligned thirds of the shard
    n_chunk = 3
    wb = [0, 16, 32, W]
    chunks = [(wb[i] * P if wb[i] * P < NSH else NSH,
               min(wb[i + 1] * P, NSH)) for i in range(n_chunk)]

    with tile.TileContext(nc) as tc:
        with (
            tc.tile_pool(name="consts", bufs=1) as cpool,
            tc.tile_pool(name="idxpool", bufs=1) as ipool,
            tc.tile_pool(name="gx1", bufs=2) as gx1pool,
            tc.tile_pool(name="oh1", bufs=2) as oh1pool,
            tc.tile_pool(name="gx2", bufs=3) as gx2pool,
            tc.tile_pool(name="oh2", bufs=2) as oh2pool,
            tc.tile_pool(name="h2loc", bufs=3) as h2lpool,
            tc.tile_pool(name="work", bufs=3) as wpool,
            tc.tile_pool(name="head", bufs=1) as hpool,
            tc.tile_pool(name="post", bufs=2) as ppool,
            tc.tile_pool(name="ps_agg", bufs=2, space="PSUM") as ps_agg,
            tc.tile_pool(name="ps_tr", bufs=1, space="PSUM") as ps_tr,
            tc.tile_pool(name="ps_h1", bufs=1, space="PSUM") as ps_h1,
            tc.tile_pool(name="ps_h2", bufs=2, space="PSUM") as ps_h2,
            tc.tile_pool(name="ps_pool", bufs=1, space="PSUM") as ps_pool,
        ):
            def load(pool, dram, shape, dt):
                t = pool.tile(shape, dt, tag=dram.name + "_sb")
                nc.sync.dma_start(out=t[:], in_=dram[tuple(slice(0, s) for s in shape)])
                return t

            negalpha = cpool.tile([P, 1], F32, tag="negalpha")
            nc.vector.memset(negalpha[:], -SELU_ALPHA)
            idx_sb = load(ipool, idxs, [P, l2.IDX_COLS], I16)
            dinv_sb = load(cpool, dinv_d, [P, W], F32)
            W1p_sb = load(cpool, W1p_d, [16, DH], BF16)
            W2_sb = load(cpool, W2_d, [P, 2 * DH], BF16)
            b1_sb = load(cpool, b1_d, [P, 2], F32)
            nb1_sb = load(cpool, nb1_d, [P, 2], F32)
            b2b_sb = load(cpool, b2b_d, [P, DH], F32)
            fc1_sb = load(cpool, fc1_d, [P, 2 * d_fc], BF16)
            fc1b_sb = load(cpool, fc1b_d, [P, 1], F32)
            nfc1b_sb = load(cpool, nfc1b_d, [P, 1], F32)
            fc2_sb = load(cpool, fc2_d, [d_fc, n_cls], BF16)
            fc2b_sb = load(cpool, fc2b_d, [P, 1], F32)
            ident_sb = load(cpool, ident_d, [P, P], F32)
            cntinv2_sb = load(cpool, cntinv2_d, [P, 2 * G], F32)

            def selu_mu(pool, z_ap, shape, out_dt, tag, bias=0.0, nbias=0.0,
                        ne_tag=None):
                """selu(z+b)/λ as two addends m = relu(z+b) and
                u = α(exp(min(z+b,0))-1); λ folded into consumer weights."""
                ne_tag = ne_tag or tag
                m = pool.tile(shape, out_dt, tag=tag + "_m")
                nc.scalar.activation(m[:], z_ap, AF.Relu, bias=bias)
                nn = pool.tile(shape, BF16, tag=ne_tag + "_n")
                nc.scalar.activation(nn[:], z_ap, AF.Relu, bias=nbias, scale=-1.0)
                e = pool.tile(shape, F32, tag=ne_tag + "_e")
                nc.scalar.activation(e[:], nn[:], AF.Exp, scale=-1.0)
                u = pool.tile(shape, out_dt, tag=tag + "_u")
                nc.scalar.activation(u[:], e[:], AF.Identity,
                                     bias=negalpha[:, 0:1], scale=SELU_ALPHA)
                return m, u

            # ---- layer-2 gather issue helper (prep-ahead overlaps phase A) ----
            h2t_lo = h2t[0:LO_SHARDS, :, :].flatten_outer_dims()
            h2t_hi = h2t[LO_SHARDS:NCORES, :, :].flatten_outer_dims()
            PREP_K = 0
            dma_sem0 = nc.alloc_semaphore("gprep0")
            dma_sem1 = nc.alloc_semaphore("gprep1")

            def issue_gathers(g, prepare):
                tl, th = l2.T_LO[g], l2.T_HI[g]
                gt2 = gx2pool.tile([P, tl + th, DH], F8, tag="gx2_t")
                nlo = tl * P
                if nlo:
                    kw = dict(prepare_only=True, sem=dma_sem0) if prepare else {}
                    nc.gpsimd.dma_gather(
                        gt2[:, 0:tl, :],
                        h2t_lo,
                        idx_sb[:, l2.lo_col[g] : l2.lo_col[g] + tl * 8],
                        nlo, nlo, DH, single_packet=False, queue_num=0, **kw,
                    )
                nhi = th * P
                if nhi:
                    kw = dict(prepare_only=True, sem=dma_sem1) if prepare else {}
                    nc.gpsimd.dma_gather(
                        gt2[:, tl : tl + th, :],
                        h2t_hi,
                        idx_sb[:, l2.hi_col[g] : l2.hi_col[g] + th * 8],
                        nhi, nhi, DH, single_packet=False, queue_num=1, **kw,
                    )
                return gt2

            gt2_prep = {}

            # ================= Phase A: layer 1 -> shard2 =================
            next_chunk = 0
            for g, (w0, wg) in enumerate(l1.groups):
                T = l1.T[g]
                gxt = gx1pool.tile([P, wg * T, 16], BF16, tag="gx1_t")
                nc.sync.dma_start(
                    out=gxt[:],
                    in_=gx1_d[:, l1.base[g] * 16 : (l1.base[g] + wg * T) * 16],
                )
                ps1g = ps_agg.tile([16, wg * P], F32, tag="ps1")
                for k in range(wg):
                    ohsl = oh1pool.tile([P, T * P], F8, tag="oh1slab")
                    nc.sync.dma_start(
                        out=ohsl[:],
                        in_=oht1_d[:, (l1.base[g] + k * T) * P
                                   : (l1.base[g] + (k + 1) * T) * P],
                    )
                    for t in range(T):
                        sl = k * T + t
                        nc.tensor.matmul(
                            ps1g[:, k * P : (k + 1) * P],
                            gxt[:, sl, :], ohsl[:, t * P : (t + 1) * P],
                            start=(t == 0), stop=(t == T - 1),
                        )
                aggxT = wpool.tile([16, wg * P], BF16, tag="aggxT")
                nc.scalar.copy(aggxT[:], ps1g[:])
                mus = []
                for j in range(2):
                    ph1g = ps_h1.tile([P, wg * P], F32, tag="ph1")
                    nc.tensor.matmul(
                        ph1g[:], W1p_sb[:, j * P : (j + 1) * P], aggxT[:],
                        start=True, stop=True,
                    )
                    m1, u1 = selu_mu(
                        ppool, ph1g[:], [P, wg * P], BF16, f"l1j{j}",
                        bias=b1_sb[:, j : j + 1] if has_b1 else 0.0,
                        nbias=nb1_sb[:, j : j + 1] if has_b1 else 0.0,
                        ne_tag="l1",
                    )
                    mus.append((m1, u1))
                for p0 in range(0, wg, 2):
                    pw = min(2, wg - p0)
                    psum_h2t = ps_h2.tile([P, pw * DH], F32, tag="main")
                    for ki in range(pw):
                        k = p0 + ki
                        for j in range(2):
                            m1, u1 = mus[j]
                            nc.tensor.matmul(
                                psum_h2t[:, ki * DH : (ki + 1) * DH],
                                m1[:, k * P : (k + 1) * P],
                                W2_sb[:, j * DH : (j + 1) * DH],
                                start=(j == 0), stop=False,
                            )
                            nc.tensor.matmul(
                                psum_h2t[:, ki * DH : (ki + 1) * DH],
                                u1[:, k * P : (k + 1) * P],
                                W2_sb[:, j * DH : (j + 1) * DH],
                                start=False, stop=(j == 1),
                            )
                    for ki in range(pw):
                        w = w0 + p0 + ki
                        h2tw = ppool.tile([P, DH], F8, tag="h2tw")
                        nc.scalar.activation(
                            h2tw[:], psum_h2t[:, ki * DH : (ki + 1) * DH],
                            AF.Copy, scale=dinv_sb[:, w : w + 1],
                        )
                        nc.sync.dma_start(
                            out=shard2[w * P : (w + 1) * P, :], in_=h2tw[:, :]
                        )
                # chunked AllGather: fire once the chunk's windows are written
                while (next_chunk < n_chunk
                       and w0 + wg >= wb[next_chunk + 1] and AG_CHUNKED):
                    a, b = chunks[next_chunk]
                    nc.gpsimd.collective_compute(
                        "AllGather", OP.bypass,
                        replica_groups=[list(range(NCORES))],
                        ins=[shard2[a:b, :]], outs=[h2t[:, a:b, :]],
                    )
                    next_chunk += 1
            if not AG_CHUNKED:
                nc.gpsimd.collective_compute(
                    "AllGather", OP.bypass,
                    replica_groups=[list(range(NCORES))],
                    ins=[shard2[0:NSH, :]], outs=[h2t[:, :, :]],
                )

            # ================= Phase B: layer 2 + pooling =================
            for g in range(PREP_K):
                gt2_prep[g] = issue_gathers(g, prepare=True)
            if PREP_K:
                nc.gpsimd.trigger_dma(count=None, queue_num=0)
                nc.gpsimd.trigger_dma(count=None, queue_num=1)
            pp0 = ps_pool.tile([P, G], F32, tag="pp0")
            pp1 = ps_pool.tile([P, G], F32, tag="pp1")
            pps = [pp0, pp1]
            for g, (w0, wg) in enumerate(l2.groups):
                gt2 = gt2_prep.pop(g) if g in gt2_prep else issue_gathers(g, False)
                TLO = l2.T_LO[g]
                ohg_sl = oh2pool.tile([P, wg * G], BF16, tag="ohg_slab")
                nc.sync.dma_start(out=ohg_sl[:], in_=ohgt_d[:, w0 * G : (w0 + wg) * G])
                sfd_sl = oh2pool.tile([P, wg * P], BF16, tag="sfd_slab")
                nc.sync.dma_start(out=sfd_sl[:], in_=selfd_d[:, w0 * P : (w0 + wg) * P])
                for p0 in range(0, wg, 2):
                    pw = min(2, wg - p0)
                    c0 = l2.colbase[g][p0]
                    k_end = p0 + pw - 1
                    c1 = (l2.colbase[g][k_end]
                          + (l2.rng_lo[g][k_end][1] - l2.rng_lo[g][k_end][0]
                             + l2.rng_hi[g][k_end][1] - l2.rng_hi[g][k_end][0]) * P)
                    ohsl = oh2pool.tile([P, c1 - c0], BF16, tag="oh2slab")
                    nc.sync.dma_start(out=ohsl[:], in_=oht2_d[:, c0:c1])
                    psum2 = ps_h2.tile([P, pw * DH], F32, tag="main")
                    for ki in range(pw):
                        k = p0 + ki
                        w = w0 + k
                        h2loc = h2lpool.tile([P, DH], F8, tag="h2loc")
                        nc.sync.dma_start(
                            out=h2loc[:], in_=shard2[w * P : (w + 1) * P, :]
                        )
                        colp = l2.colbase[g][k] - c0
                        first = True
                        for (t0, t1), sbase in ((l2.rng_lo[g][k], 0),
                                                (l2.rng_hi[g][k], TLO)):
                            for t in range(t0, t1):
                                nc.tensor.matmul(
                                    psum2[:, ki * DH : (ki + 1) * DH],
                                    ohsl[:, colp : colp + P],
                                    gt2[:, sbase + t, :],
                                    start=first, stop=False,
                                )
                                first = False
                                colp += P
                        nc.tensor.matmul(
                            psum2[:, ki * DH : (ki + 1) * DH],
                            sfd_sl[:, k * P : (k + 1) * P], h2loc[:],
                            start=False, stop=True,
                        )
                    if has_b2:
                        zdp = ppool.tile([P, pw * DH], F32, tag="l2_zd")
                        for ki in range(pw):
                            nc.vector.tensor_tensor(
                                zdp[:, ki * DH : (ki + 1) * DH],
                                psum2[:, ki * DH : (ki + 1) * DH], b2b_sb[:], OP.add)
                        m2, u2 = selu_mu(ppool, zdp[:], [P, pw * DH], BF16, "l2")
                    else:
                        m2, u2 = selu_mu(ppool, psum2[:], [P, pw * DH], BF16, "l2")
                    for ki in range(pw):
                        k = p0 + ki
                        w = w0 + k
                        for j in range(2):
                            for part in (m2, u2):
                                nc.tensor.matmul(
                                    pps[j][:],
                                    part[:, ki * DH + j * P : ki * DH + (j + 1) * P],
                                    ohg_sl[:, k * G : (k + 1) * G],
                                    start=(w == 0 and part is m2),
                                    stop=(w == W - 1 and part is u2),
                                )

            # ================= pooled head =================
            pT = hpool.tile([P, 2 * G], F32, tag="pT")
            nc.scalar.copy(pT[:, 0:G], pp0[:])
            nc.scalar.copy(pT[:, G : 2 * G], pp1[:])
            nc.sync.dma_start(out=pool_part[0:P, :], in_=pT[:, 0:G])
            nc.sync.dma_start(out=pool_part[P : 2 * P, :], in_=pT[:, G : 2 * G])
            nc.gpsimd.collective_compute(
                "AllReduce", OP.add,
                replica_groups=[list(range(NCORES))],
                ins=[pool_part[:, :]], outs=[pool_sum[:, :]],
            )
            ps = hpool.tile([P, 2 * G], F32, tag="ps_in")
            nc.sync.dma_start(out=ps[:, 0:G], in_=pool_sum[0:P, :])
            nc.sync.dma_start(out=ps[:, G : 2 * G], in_=pool_sum[P : 2 * P, :])
            pm = hpool.tile([P, 2 * G], F32, tag="pm")
            nc.vector.tensor_tensor(pm[:], ps[:], cntinv2_sb[:], OP.mult)
            gm, gu = selu_mu(hpool, pm[:], [P, 2 * G], BF16, "hd1")

            psum_fc1 = ps_h2.tile([P, G], F32, tag="main")
            for j in range(2):
                for pi, part in enumerate((gm, gu)):
                    nc.tensor.matmul(
                        psum_fc1[:], fc1_sb[:, j * d_fc : (j + 1) * d_fc],
                        part[:, j * G : (j + 1) * G],
                        start=(j == 0 and pi == 0), stop=(j == 1 and pi == 1),
                    )
            hm, hu = selu_mu(hpool, psum_fc1[:], [P, G], BF16, "hd2",
                             bias=fc1b_sb[:, 0:1], nbias=nfc1b_sb[:, 0:1])

            psum_fc2 = ps_tr.tile([n_cls, G], F32, tag="sm")
            nc.tensor.matmul(psum_fc2[:], fc2_sb[:], hm[:], start=True, stop=False)
            nc.tensor.matmul(psum_fc2[:], fc2_sb[:], hu[:], start=False, stop=True)
            lg2 = wpool.tile([n_cls, G], F32, tag="lg2")
            nc.scalar.activation(
                lg2[:], psum_fc2[:], AF.Identity, bias=fc2b_sb[0:n_cls, 0:1]
            )
            for j in range(-(-G // P)):
                gw = min(P, G - j * P)
                psT2 = ps_tr.tile([P, n_cls], F32, tag="sm")
                nc.tensor.transpose(
                    psT2[:gw, :], lg2[:, j * P : j * P + gw],
                    ident_sb[0:n_cls, 0:n_cls],
                )
                lgj = hpool.tile([P, n_cls], F32, tag="lgj")
                nc.scalar.copy(lgj[:gw, :], psT2[:gw, :])
                nm = hpool.tile([P, 1], F32, tag="nm")
                nc.vector.tensor_reduce(
                    nm[:gw, :], lgj[:gw, :], mybir.AxisListType.X, OP.max, negate=True
                )
                e4 = hpool.tile([P, n_cls], F32, tag="e4")
                nc.scalar.activation(e4[:gw, :], lgj[:gw, :], AF.Exp, bias=nm[:gw, 0:1])
                s4 = hpool.tile([P, 1], F32, tag="s4")
                nc.vector.tensor_reduce(s4[:gw, :], e4[:gw, :], mybir.AxisListType.X, OP.add)
                ls = hpool.tile([P, 1], F32, tag="ls")
                nc.scalar.activation(ls[:gw, :], s4[:gw, :], AF.Ln)
                q = hpool.tile([P, 1], F32, tag="q")
                nc.vector.tensor_tensor(q[:gw, :], nm[:gw, :], ls[:gw, :], OP.subtract)
                outj = hpool.tile([P, n_cls], F32, tag="outj")
                nc.vector.tensor_scalar(outj[:gw, :], lgj[:gw, :], q[:gw, 0:1], None, OP.add)
                nc.sync.dma_start(out=out_d[j * P : j * P + gw, :], in_=outj[:gw, :])

    nc.compile()
    return nc


_CACHE = {}


def run_gcn(inputs, n_nodes, n_graphs, d_in=14, d_hid=256, d_fc=128, n_cls=2,
            grp1=4, grp2=8, trace=False):
    split = LO_SHARDS * (n_nodes // NCORES)
    s, d, cut, nlo, nhi, cnt1 = edge_partition(inputs, n_nodes, split)
    l1 = Lay1(n_nodes, cnt1, grp1)
    l2 = Lay2(n_nodes, nlo, nhi, grp2)
    per_core, shared = host_prep(inputs, s, d, cut, l1, l2, n_nodes, n_graphs, split)
    key = (n_nodes, n_graphs, tuple(l1.T), tuple(l2.T_LO), tuple(l2.T_HI),
           grp1, grp2, shared["has_b1"], shared["has_b2"])
    if key not in _CACHE:
        _CACHE[key] = build_nc(l1, l2, n_nodes, n_graphs, d_hid, d_fc, n_cls,
                               shared["has_b1"], shared["has_b2"])
    nc = _CACHE[key]
    res = run_bass_kernel_spmd(nc, per_core, list(range(NCORES)), trace=trace)
    return res.results[0]["out"].astype(np.float32), res


def kernel(**inputs) -> np.ndarray:
    out, _ = run_gcn(
        inputs, n_nodes=50000, n_graphs=256,
        trace=bool(int(os.environ.get("GCN_TRACE", "0"))),
    )
    return out


# revision 30
# speedup vs baseline: 1.2222x; 1.2222x over previous
"""Trainium2 Bass kernel for a 2-layer GCN + global mean pool + MLP head.

Strategy (8 NeuronCores, SPMD):
  - Nodes (and their incident edges, grouped by destination) are sharded
    across the 8 cores; each core owns N/8 destination nodes.
  - Layer 1's edge gather is done ON THE HOST (x is an input): each core
    receives a pre-expanded [128, slots, 16] bf16 stream of
    x[src] * dinv[src] * dinv[dst] values (self-loops included), so the
    device does zero gather work for layer 1.  Aggregation is a one-hot
    matmul per 128-edge tile directly into a transposed [16, wg*128] PSUM
    batched over the whole window group.
  - Layer 2 gathers rows of the AllGather'ed (h1 @ W2) * dinv table with
    gpsimd dma_gather (dst-sorted edges, lo/hi split for int16 indices).
    The AllGather is chunked so it overlaps the tail of layer 1.
    One-hot values carry dinv[dst] (bf16), so the aggregation PSUM needs
    no per-window scale and SELU batches across window pairs.
    Self-loops are a diag(dinv[dst]) stationary against the local shard.
  - SELU is computed as m + alpha*(exp(min(z,0))-1) via Relu/Exp on the
    scalar engine; m and u feed the same accumulating matmul so no add is
    needed, and lambda is folded into the next weights host-side.
  - Mean-pool partial sums use one-hot-matmul (node -> graph id),
    AllReduce-summed; the tiny MLP head + log_softmax run redundantly.
"""

import os
import numpy as np
import ml_dtypes

import concourse.bacc as bacc
import concourse.bass as bass
import concourse.mybir as mybir
import concourse.tile as tile
from concourse.bass_utils import run_bass_kernel_spmd

F32 = mybir.dt.float32
F8 = mybir.dt.float8e4
BF16 = mybir.dt.bfloat16
I16 = mybir.dt.int16
AF = mybir.ActivationFunctionType
OP = mybir.AluOpType
NPBF16 = ml_dtypes.bfloat16
NPF8 = ml_dtypes.float8_e4m3

SELU_LAM = 1.0507009873554805
SELU_ALPHA = 1.6732632423543772

P = 128
NCORES = 8
AG_CHUNKED = False
LO_SHARDS = 5  # table split for int16 gather indices: lo = first 5 shards


def _groups(W, grp):
    out = []
    w = 0
    while w < W:
        wg = min(grp, W - w)
        out.append((w, wg))
        w += wg
    return out


class Lay1:
    """Layer-1 host-expanded layout: window-major dense slots per group."""

    def __init__(self, n_nodes, cnt1_cw, grp):
        self.NSH = n_nodes // NCORES
        self.W = -(-self.NSH // P)
        self.groups = _groups(self.W, grp)
        self.T = []
        self.base = []
        b = 0
        for (w0, wg) in self.groups:
            t = max(
                -(-int(cnt1_cw[c, w]) // P)
                for c in range(NCORES)
                for w in range(w0, w0 + wg)
            )
            self.T.append(t)
            self.base.append(b)
            b += wg * t
        self.S_TOT = b


class Lay2:
    """Layer-2 gather layout: group-contiguous lo/hi int16 index streams.

    Edges of a window group are packed back-to-back (window-major) into one
    lo and one hi stream per group; only the stream tail is padded.  Each
    window's edges then span a *static* tile range (min/max over cores of
    its per-core prefix offsets); boundary tiles shared by two windows get
    one matmul (with a window-masked one-hot block) per window."""

    def __init__(self, n_nodes, nlo_cw, nhi_cw, grp):
        self.NSH = n_nodes // NCORES
        self.W = -(-self.NSH // P)
        self.groups = _groups(self.W, grp)
        self.T_LO, self.T_HI = [], []
        self.rng_lo, self.rng_hi = [], []   # per group: list per window (t0, t1)
        self.colbase = []                    # per group: per window, per stream col base
        self.lo_col, self.hi_col = [], []    # idx slab col offsets
        self.slot_base = []                  # gt2 slot base per group (0; per-group tiles)
        col = 0
        ohcol = 0
        for g, (w0, wg) in enumerate(self.groups):
            lo_off = np.zeros((NCORES, wg + 1), np.int64)
            hi_off = np.zeros((NCORES, wg + 1), np.int64)
            for c in range(NCORES):
                lo_off[c, 1:] = np.cumsum(nlo_cw[c, w0 : w0 + wg])
                hi_off[c, 1:] = np.cumsum(nhi_cw[c, w0 : w0 + wg])
            tl = int(max(-(-lo_off[c, wg] // P) for c in range(NCORES)))
            th = int(max(-(-hi_off[c, wg] // P) for c in range(NCORES)))
            self.T_LO.append(tl)
            self.T_HI.append(th)
            rl, rh, cb = [], [], []
            for k in range(wg):
                lt0 = int(min(lo_off[c, k] // P for c in range(NCORES)))
                lt1 = int(max(-(-lo_off[c, k + 1] // P) for c in range(NCORES)))
                ht0 = int(min(hi_off[c, k] // P for c in range(NCORES)))
                ht1 = int(max(-(-hi_off[c, k + 1] // P) for c in range(NCORES)))
                rl.append((lt0, lt1))
                rh.append((ht0, ht1))
                cb.append(ohcol)
                ohcol += (lt1 - lt0 + ht1 - ht0) * P
            self.rng_lo.append(rl)
            self.rng_hi.append(rh)
            self.colbase.append(cb)
            self.lo_col.append(col)
            col += tl * 8
            self.hi_col.append(col)
            col += th * 8
        self.IDX_COLS = col
        self.OH_COLS = ohcol
        self.lo_off_all = None  # filled by host_prep


def edge_partition(inputs, n_nodes, split):
    """Sort edges by destination; per-(core,window) counts (no self-loops)."""
    ei = np.asarray(inputs["edge_index"], np.int64)
    src, dst = ei[0], ei[1]
    order = np.argsort(dst, kind="stable")
    s, d = src[order], dst[order]
    nsh = n_nodes // NCORES
    W = -(-nsh // P)
    bounds = [c * nsh + w * P for c in range(NCORES) for w in range(W)] + [n_nodes]
    cut = np.searchsorted(d, np.asarray(bounds))
    nlo = np.zeros((NCORES, W), np.int64)
    nhi = np.zeros((NCORES, W), np.int64)
    cnt1 = np.zeros((NCORES, W), np.int64)
    for i in range(NCORES * W):
        sw = s[cut[i] : cut[i + 1]]
        c, w = i // W, i % W
        rows = min(P, nsh - w * P)
        l = int((sw < split).sum())
        nlo[c, w] = l
        nhi[c, w] = len(sw) - l
        cnt1[c, w] = len(sw) + rows  # + self-loops
    return s, d, cut, nlo, nhi, cnt1


def host_prep(inputs, s, d, cut, l1, l2, n_nodes, n_graphs, split):
    N, G = n_nodes, n_graphs
    W = l1.W
    NSH = l1.NSH
    x = np.asarray(inputs["x"], np.float32)
    batch = np.asarray(inputs["batch"], np.int64)
    D_IN = x.shape[1]

    deg = np.bincount(d, minlength=N).astype(np.float64) + 1.0  # + self loop
    dinv = (1.0 / np.sqrt(deg)).astype(np.float32)
    xs = (x * dinv[:, None]).astype(np.float32)

    cnt = np.bincount(batch, minlength=G).astype(np.float32)
    cntinv = (SELU_LAM / np.maximum(cnt, 1.0)).astype(np.float32)  # λ2 folded

    per_core = []
    for c in range(NCORES):
        # ---------- layer 1: host-expanded values + one-hots ----------
        gx1 = np.zeros((l1.S_TOT * P, 16), np.float32)
        dl1 = np.full((P, l1.S_TOT), -1.0, np.float32)
        for g, (w0, wg) in enumerate(l1.groups):
            T = l1.T[g]
            for k in range(wg):
                w = w0 + k
                i = c * W + w
                sw = s[cut[i] : cut[i + 1]]
                dw = d[cut[i] : cut[i + 1]] - (c * NSH + w * P)
                rows = min(P, NSH - w * P)
                base = c * NSH + w * P
                srcs = np.concatenate([sw, np.arange(base, base + rows)])
                dsts = np.concatenate([dw, np.arange(rows)]).astype(np.int64)
                ddst = dinv[c * NSH + w * P + dsts]
                n_e = len(srcs)
                slot0 = l1.base[g] + k * T
                gx1[slot0 * P : slot0 * P + n_e, :D_IN] = xs[srcs, :D_IN] * ddst[:, None]
                flat = np.full(T * P, -1.0, np.float32)
                flat[:n_e] = dsts
                dl1[:, slot0 : slot0 + T] = flat.reshape(T, P).T
        oh1 = (dl1[:, :, None] == np.arange(P, dtype=np.float32)[None, None, :])
        oh1 = oh1.astype(NPF8).reshape(P, l1.S_TOT * P)
        gx1v = gx1.reshape(l1.S_TOT, P, 16).transpose(1, 0, 2).reshape(P, l1.S_TOT * 16)
        gx1v = gx1v.astype(NPBF16)

        # per-window dst dinv / graph one-hot
        dinv_w = np.zeros((P, W), np.float32)
        batchloc = np.full((P, W), -1.0, np.float32)
        base = c * NSH
        for w in range(W):
            rows = min(P, NSH - w * P)
            dinv_w[:rows, w] = dinv[base + w * P : base + w * P + rows]
            batchloc[:rows, w] = batch[base + w * P : base + w * P + rows].astype(np.float32)
        ohg = (batchloc[:, :, None] == np.arange(G, dtype=np.float32)[None, None, :])
        ohg = ohg.astype(NPBF16).reshape(P, W * G)

        # ---------- layer 2: group-contiguous idx streams + one-hots ----------
        idx_slab = np.zeros((16, l2.IDX_COLS), np.int16)
        oh2 = np.zeros((P, l2.OH_COLS), np.float32)
        for g, (w0, wg) in enumerate(l2.groups):
            lo_lists, hi_lists, lo_d, hi_d = [], [], [], []
            for k in range(wg):
                w = w0 + k
                i = c * W + w
                sw = s[cut[i] : cut[i + 1]]
                dw = (d[cut[i] : cut[i + 1]] - (c * NSH + w * P)).astype(np.int64)
                m = sw < split
                lo_lists.append(sw[m]); lo_d.append(dw[m])
                hi_lists.append(sw[~m] - split); hi_d.append(dw[~m])
            for lists, dls, rng, tl, col0 in (
                (lo_lists, lo_d, l2.rng_lo[g], l2.T_LO[g], l2.lo_col[g]),
                (hi_lists, hi_d, l2.rng_hi[g], l2.T_HI[g], l2.hi_col[g]),
            ):
                flat = np.concatenate(lists) if lists else np.zeros(0, np.int64)
                st = np.zeros(tl * P, np.int16)
                st[: len(flat)] = flat.astype(np.int16)
                idx_slab[:, col0 : col0 + tl * 8] = st.reshape(-1, 16).T
            # one-hot blocks per (window, stream, tile)
            lo_off = np.concatenate([[0], np.cumsum([len(x) for x in lo_lists])])
            hi_off = np.concatenate([[0], np.cumsum([len(x) for x in hi_lists])])
            lo_flat_d = np.concatenate(lo_d) if lo_d else np.zeros(0, np.int64)
            hi_flat_d = np.concatenate(hi_d) if hi_d else np.zeros(0, np.int64)
            for k in range(wg):
                w = w0 + k
                colp = l2.colbase[g][k]
                for (t0, t1), off, fd in (
                    (l2.rng_lo[g][k], lo_off, lo_flat_d),
                    (l2.rng_hi[g][k], hi_off, hi_flat_d),
                ):
                    for t in range(t0, t1):
                        p0, p1 = t * P, (t + 1) * P
                        a = max(p0, int(off[k])); b = min(p1, int(off[k + 1]))
                        if b > a:
                            rows = np.arange(a - p0, b - p0)
                            dl = fd[a:b]
                            oh2[rows, colp + dl] = dinv_w[dl, w]
                        colp += P
        oh2 = oh2.astype(NPBF16)

        # self-loop stationary: diag(dinv[d]) per window, bf16
        selfd = np.zeros((P, W * P), NPBF16)
        for w in range(W):
            selfd[:, w * P : (w + 1) * P][np.arange(P), np.arange(P)] = dinv_w[:, w].astype(NPBF16)

        per_core.append({
            "gx1": gx1v,
            "oht1": oh1,
            "idxs": np.tile(idx_slab, (8, 1)),
            "oht2": oh2,
            "selfd": selfd,
            "ohgt": ohg,
            "dinv_w": dinv_w,
        })

    # ---------- shared constants (SELU lambdas folded downstream) ----------
    D_HID = np.asarray(inputs["W1"]).shape[1]
    W1p = np.zeros((16, D_HID), NPBF16)
    W1p[:D_IN] = np.asarray(inputs["W1"], np.float32).astype(NPBF16)
    W2 = np.asarray(inputs["W2"], np.float32) * SELU_LAM  # λ1
    W2_sb = np.concatenate([W2[:P, :], W2[P:, :]], axis=1).astype(NPBF16)
    b1 = np.asarray(inputs["b1"], np.float32).reshape(2, P).T.copy()
    b2b = np.tile(np.asarray(inputs["b2"], np.float32)[None, :], (P, 1))
    fc1 = np.asarray(inputs["fc1_w"], np.float32) * SELU_LAM  # λ3
    fc1_sb = np.concatenate([fc1[:P, :], fc1[P:, :]], axis=1).astype(NPBF16)
    fc1b = np.asarray(inputs["fc1_b"], np.float32).reshape(P, 1)
    fc2 = (np.asarray(inputs["fc2_w"], np.float32) * SELU_LAM).astype(NPBF16)  # λ4
    N_CLS = fc2.shape[1]
    fc2b = np.zeros((P, 1), np.float32)
    fc2b[:N_CLS, 0] = np.asarray(inputs["fc2_b"], np.float32)
    ident = np.eye(P, dtype=np.float32)
    cntinv2 = np.tile(cntinv[None, :], (P, 2))

    shared = {
        "W1p": W1p,
        "W2_sb": W2_sb,
        "b1h": b1,
        "nb1h": -b1,
        "b2b": b2b,
        "fc1_sb": fc1_sb,
        "fc1b": fc1b,
        "nfc1b": -fc1b,
        "fc2_sb": fc2,
        "fc2b": fc2b,
        "ident": ident,
        "cntinv2": cntinv2,
        "has_b1": bool(np.any(b1)),
        "has_b2": bool(np.any(b2b)),
    }
    for im in per_core:
        for k, v in shared.items():
            if not k.startswith("has_"):
                im[k] = v
    return per_core, shared


def build_nc(l1, l2, n_nodes, n_graphs, d_hid, d_fc, n_cls, has_b1, has_b2):
    nc = bacc.Bacc("TRN2", target_bir_lowering=False, debug=False,
                   num_devices=NCORES, num_swdge_queues=2)
    N, G, W = n_nodes, n_graphs, l1.W
    NSH = l1.NSH
    DH = d_hid
    SH2 = W * P
    SPLIT = LO_SHARDS * NSH

    gx1_d = nc.dram_tensor("gx1", [P, l1.S_TOT * 16], BF16, kind="ExternalInput")
    oht1_d = nc.dram_tensor("oht1", [P, l1.S_TOT * P], F8, kind="ExternalInput")
    idxs = nc.dram_tensor("idxs", [P, l2.IDX_COLS], I16, kind="ExternalInput")
    oht2_d = nc.dram_tensor("oht2", [P, l2.OH_COLS], BF16, kind="ExternalInput")
    selfd_d = nc.dram_tensor("selfd", [P, W * P], BF16, kind="ExternalInput")
    dinv_d = nc.dram_tensor("dinv_w", [P, W], F32, kind="ExternalInput")
    ohgt_d = nc.dram_tensor("ohgt", [P, W * G], BF16, kind="ExternalInput")
    W1p_d = nc.dram_tensor("W1p", [16, DH], BF16, kind="ExternalInput")
    W2_d = nc.dram_tensor("W2_sb", [P, 2 * DH], BF16, kind="ExternalInput")
    b1_d = nc.dram_tensor("b1h", [P, 2], F32, kind="ExternalInput")
    nb1_d = nc.dram_tensor("nb1h", [P, 2], F32, kind="ExternalInput")
    b2b_d = nc.dram_tensor("b2b", [P, DH], F32, kind="ExternalInput")
    fc1_d = nc.dram_tensor("fc1_sb", [P, 2 * d_fc], BF16, kind="ExternalInput")
    fc1b_d = nc.dram_tensor("fc1b", [P, 1], F32, kind="ExternalInput")
    nfc1b_d = nc.dram_tensor("nfc1b", [P, 1], F32, kind="ExternalInput")
    fc2_d = nc.dram_tensor("fc2_sb", [d_fc, n_cls], BF16, kind="ExternalInput")
    fc2b_d = nc.dram_tensor("fc2b", [P, 1], F32, kind="ExternalInput")
    ident_d = nc.dram_tensor("ident", [P, P], F32, kind="ExternalInput")
    cntinv2_d = nc.dram_tensor("cntinv2", [P, 2 * G], F32, kind="ExternalInput")

    out_d = nc.dram_tensor("out", [G, n_cls], F32, kind="ExternalOutput")

    shard2 = nc.dram_tensor("shard2", [SH2, DH], F8)
    h2t = nc.dram_tensor("h2t", [NCORES, NSH, DH], F8, addr_space="Shared")
    pool_part = nc.dram_tensor("pool_part", [2 * P, G], F32)
    pool_sum = nc.dram_tensor("pool_sum", [2 * P, G], F32, addr_space="Shared")

    # AllGather chunks: window-aligned thirds of the shard
    n_chunk = 3
    wb = [0, 16, 32, W]
    chunks = [(wb[i] * P if wb[i] * P < NSH else NSH,
               min(wb[i + 1] * P, NSH)) for i in range(n_chunk)]

    with tile.TileContext(nc) as tc:
        with (
            tc.tile_pool(name="consts", bufs=1) as cpool,
            tc.tile_pool(name="idxpool", bufs=1) as ipool,
            tc.tile_pool(name="gx1", bufs=2) as gx1pool,
            tc.tile_pool(name="oh1", bufs=2) as oh1pool,
            tc.tile_pool(name="gx2", bufs=3) as gx2pool,
            tc.tile_pool(name="oh2", bufs=2) as oh2pool,
            tc.tile_pool(name="h2loc", bufs=3) as h2lpool,
            tc.tile_pool(name="work", bufs=3) as wpool,
            tc.tile_pool(name="head", bufs=1) as hpool,
            tc.tile_pool(name="post", bufs=2) as ppool,
            tc.tile_pool(name="ps_agg", bufs=2, space="PSUM") as ps_agg,
            tc.tile_pool(name="ps_tr", bufs=1, space="PSUM") as ps_tr,
            tc.tile_pool(name="ps_h1", bufs=1, space="PSUM") as ps_h1,
            tc.tile_pool(name="ps_h2", bufs=2, space="PSUM") as ps_h2,
            tc.tile_pool(name="ps_pool", bufs=1, space="PSUM") as ps_pool,
        ):
            def load(pool, dram, shape, dt):
                t = pool.tile(shape, dt, tag=dram.name + "_sb")
                nc.sync.dma_start(out=t[:], in_=dram[tuple(slice(0, s) for s in shape)])
                return t

            negalpha = cpool.tile([P, 1], F32, tag="negalpha")
            nc.vector.memset(negalpha[:], -SELU_ALPHA)
            idx_sb = load(ipool, idxs, [P, l2.IDX_COLS], I16)
            dinv_sb = load(cpool, dinv_d, [P, W], F32)
            W1p_sb = load(cpool, W1p_d, [16, DH], BF16)
            W2_sb = load(cpool, W2_d, [P, 2 * DH], BF16)
            b1_sb = load(cpool, b1_d, [P, 2], F32)
            nb1_sb = load(cpool, nb1_d, [P, 2], F32)
            b2b_sb = load(cpool, b2b_d, [P, DH], F32)
            fc1_sb = load(cpool, fc1_d, [P, 2 * d_fc], BF16)
            fc1b_sb = load(cpool, fc1b_d, [P, 1], F32)
            nfc1b_sb = load(cpool, nfc1b_d, [P, 1], F32)
            fc2_sb = load(cpool, fc2_d, [d_fc, n_cls], BF16)
            fc2b_sb = load(cpool, fc2b_d, [P, 1], F32)
            ident_sb = load(cpool, ident_d, [P, P], F32)
            cntinv2_sb = load(cpool, cntinv2_d, [P, 2 * G], F32)

            def selu_mu(pool, z_ap, shape, out_dt, tag, bias=0.0, nbias=0.0,
                        ne_tag=None):
                """selu(z+b)/λ as two addends m = relu(z+b) and
                u = α(exp(min(z+b,0))-1); λ folded into consumer weights."""
                ne_tag = ne_tag or tag
                m = pool.tile(shape, out_dt, tag=tag + "_m")
                nc.scalar.activation(m[:], z_ap, AF.Relu, bias=bias)
                nn = pool.tile(shape, BF16, tag=ne_tag + "_n")
                nc.scalar.activation(nn[:], z_ap, AF.Relu, bias=nbias, scale=-1.0)
                e = pool.tile(shape, F32, tag=ne_tag + "_e")
                nc.scalar.activation(e[:], nn[:], AF.Exp, scale=-1.0)
                u = pool.tile(shape, out_dt, tag=tag + "_u")
                nc.scalar.activation(u[:], e[:], AF.Identity,
                                     bias=negalpha[:, 0:1], scale=SELU_ALPHA)
                return m, u

            # ---- layer-2 gather issue helper (prep-ahead overlaps phase A) ----
            h2t_lo = h2t[0:LO_SHARDS, :, :].flatten_outer_dims()
            h2t_hi = h2t[LO_SHARDS:NCORES, :, :].flatten_outer_dims()
            PREP_K = 0
            dma_sem0 = nc.alloc_semaphore("gprep0")
            dma_sem1 = nc.alloc_semaphore("gprep1")

            def issue_gathers(g, prepare):
                tl, th = l2.T_LO[g], l2.T_HI[g]
                gt2 = gx2pool.tile([P, tl + th, DH], F8, tag="gx2_t")
                nlo = tl * P
                if nlo:
                    kw = dict(prepare_only=True, sem=dma_sem0) if prepare else {}
                    nc.gpsimd.dma_gather(
                        gt2[:, 0:tl, :],
                        h2t_lo,
                        idx_sb[:, l2.lo_col[g] : l2.lo_col[g] + tl * 8],
                        nlo, nlo, DH, single_packet=False, queue_num=0, **kw,
                    )
                nhi = th * P
                if nhi:
                    kw = dict(prepare_only=True, sem=dma_sem1) if prepare else {}
                    nc.gpsimd.dma_gather(
                        gt2[:, tl : tl + th, :],
                        h2t_hi,
                        idx_sb[:, l2.hi_col[g] : l2.hi_col[g] + th * 8],
                        nhi, nhi, DH, single_packet=False, queue_num=1, **kw,
                    )
                return gt2

            gt2_prep = {}

            # ================= Phase A: layer 1 -> shard2 =================
            next_chunk = 0
            for g, (w0, wg) in enumerate(l1.groups):
                T = l1.T[g]
                gxt = gx1pool.tile([P, wg * T, 16], BF16, tag="gx1_t")
                nc.sync.dma_start(
                    out=gxt[:],
                    in_=gx1_d[:, l1.base[g] * 16 : (l1.base[g] + wg * T) * 16],
                )
                ps1g = ps_agg.tile([16, wg * P], F32, tag="ps1")
                for k in range(wg):
                    ohsl = oh1pool.tile([P, T * P], F8, tag="oh1slab")
                    nc.sync.dma_start(
                        out=ohsl[:],
                        in_=oht1_d[:, (l1.base[g] + k * T) * P
                                   : (l1.base[g] + (k + 1) * T) * P],
                    )
                    for t in range(T):
                        sl = k * T + t
                        nc.tensor.matmul(
                            ps1g[:, k * P : (k + 1) * P],
                            gxt[:, sl, :], ohsl[:, t * P : (t + 1) * P],
                            start=(t == 0), stop=(t == T - 1),
                        )
                aggxT = wpool.tile([16, wg * P], BF16, tag="aggxT")
                nc.scalar.copy(aggxT[:], ps1g[:])
                mus = []
                for j in range(2):
                    ph1g = ps_h1.tile([P, wg * P], F32, tag="ph1")
                    nc.tensor.matmul(
                        ph1g[:], W1p_sb[:, j * P : (j + 1) * P], aggxT[:],
                        start=True, stop=True,
                    )
                    m1, u1 = selu_mu(
                        ppool, ph1g[:], [P, wg * P], BF16, f"l1j{j}",
                        bias=b1_sb[:, j : j + 1] if has_b1 else 0.0,
                        nbias=nb1_sb[:, j : j + 1] if has_b1 else 0.0,
                        ne_tag="l1",
                    )
                    mus.append((m1, u1))
                for p0 in range(0, wg, 2):
                    pw = min(2, wg - p0)
                    psum_h2t = ps_h2.tile([P, pw * DH], F32, tag="main")
                    for ki in range(pw):
                        k = p0 + ki
                        for j in range(2):
                            m1, u1 = mus[j]
                            nc.tensor.matmul(
                                psum_h2t[:, ki * DH : (ki + 1) * DH],
                                m1[:, k * P : (k + 1) * P],
                                W2_sb[:, j * DH : (j + 1) * DH],
                                start=(j == 0), stop=False,
                            )
                            nc.tensor.matmul(
                                psum_h2t[:, ki * DH : (ki + 1) * DH],
                                u1[:, k * P : (k + 1) * P],
                                W2_sb[:, j * DH : (j + 1) * DH],
                                start=False, stop=(j == 1),
                            )
                    for ki in range(pw):
                        w = w0 + p0 + ki
                        h2tw = ppool.tile([P, DH], F8, tag="h2tw")
                        nc.scalar.activation(
                            h2tw[:], psum_h2t[:, ki * DH : (ki + 1) * DH],
                            AF.Copy, scale=dinv_sb[:, w : w + 1],
                        )
                        nc.sync.dma_start(
                            out=shard2[w * P : (w + 1) * P, :], in_=h2tw[:, :]
                        )
                # chunked AllGather: fire once the chunk's windows are written
                while (next_chunk < n_chunk
                       and w0 + wg >= wb[next_chunk + 1] and AG_CHUNKED):
                    a, b = chunks[next_chunk]
                    nc.gpsimd.collective_compute(
                        "AllGather", OP.bypass,
                        replica_groups=[list(range(NCORES))],
                        ins=[shard2[a:b, :]], outs=[h2t[:, a:b, :]],
                    )
                    next_chunk += 1
            if not AG_CHUNKED:
                nc.gpsimd.collective_compute(
                    "AllGather", OP.bypass,
                    replica_groups=[list(range(NCORES))],
                    ins=[shard2[0:NSH, :]], outs=[h2t[:, :, :]],
                )

            # ================= Phase B: layer 2 + pooling =================
            for g in range(PREP_K):
                gt2_prep[g] = issue_gathers(g, prepare=True)
            if PREP_K:
                nc.gpsimd.trigger_dma(count=None, queue_num=0)
                nc.gpsimd.trigger_dma(count=None, queue_num=1)
            pp0 = ps_pool.tile([P, G], F32, tag="pp0")
            pp1 = ps_pool.tile([P, G], F32, tag="pp1")
            pps = [pp0, pp1]
            for g, (w0, wg) in enumerate(l2.groups):
                gt2 = gt2_prep.pop(g) if g in gt2_prep else issue_gathers(g, False)
                TLO = l2.T_LO[g]
                ohg_sl = oh2pool.tile([P, wg * G], BF16, tag="ohg_slab")
                nc.sync.dma_start(out=ohg_sl[:], in_=ohgt_d[:, w0 * G : (w0 + wg) * G])
                sfd_sl = oh2pool.tile([P, wg * P], BF16, tag="sfd_slab")
                nc.sync.dma_start(out=sfd_sl[:], in_=selfd_d[:, w0 * P : (w0 + wg) * P])
                for p0 in range(0, wg, 2):
                    pw = min(2, wg - p0)
                    c0 = l2.colbase[g][p0]
                    k_end = p0 + pw - 1
                    c1 = (l2.colbase[g][k_end]
                          + (l2.rng_lo[g][k_end][1] - l2.rng_lo[g][k_end][0]
                             + l2.rng_hi[g][k_end][1] - l2.rng_hi[g][k_end][0]) * P)
                    ohsl = oh2pool.tile([P, c1 - c0], BF16, tag="oh2slab")
                    nc.sync.dma_start(out=ohsl[:], in_=oht2_d[:, c0:c1])
                    psum2 = ps_h2.tile([P, pw * DH], F32, tag="main")
                    for ki in range(pw):
                        k = p0 + ki
                        w = w0 + k
                        h2loc = h2lpool.tile([P, DH], F8, tag="h2loc")
                        nc.sync.dma_start(
                            out=h2loc[:], in_=shard2[w * P : (w + 1) * P, :]
                        )
                        colp = l2.colbase[g][k] - c0
                        first = True
                        for (t0, t1), sbase in ((l2.rng_lo[g][k], 0),
                                                (l2.rng_hi[g][k], TLO)):
                            for t in range(t0, t1):
                                nc.tensor.matmul(
                                    psum2[:, ki * DH : (ki + 1) * DH],
                                    ohsl[:, colp : colp + P],
                                    gt2[:, sbase + t, :],
                                    start=first, stop=False,
                                )
                                first = False
                                colp += P
                        nc.tensor.matmul(
                            psum2[:, ki * DH : (ki + 1) * DH],
                            sfd_sl[:, k * P : (k + 1) * P], h2loc[:],
                            start=False, stop=True,
                        )
                    if has_b2:
                        zdp = ppool.tile([P, pw * DH], F32, tag="l2_zd")
                        for ki in range(pw):
                            nc.vector.tensor_tensor(
                                zdp[:, ki * DH : (ki + 1) * DH],
                                psum2[:, ki * DH : (ki + 1) * DH], b2b_sb[:], OP.add)
                        m2, u2 = selu_mu(ppool, zdp[:], [P, pw * DH], BF16, "l2")
                    else:
                        m2, u2 = selu_mu(ppool, psum2[:], [P, pw * DH], BF16, "l2")
                    for ki in range(pw):
                        k = p0 + ki
                        w = w0 + k
                        for j in range(2):
                            for part in (m2, u2):
                                nc.tensor.matmul(
                                    pps[j][:],
                                    part[:, ki * DH + j * P : ki * DH + (j + 1) * P],
                                    ohg_sl[:, k * G : (k + 1) * G],
                                    start=(w == 0 and part is m2),
                                    stop=(w == W - 1 and part is u2),
                                )

            # ================= pooled head =================
            pT = hpool.tile([P, 2 * G], F32, tag="pT")
            nc.scalar.copy(pT[:, 0:G], pp0[:])
            nc.scalar.copy(pT[:, G : 2 * G], pp1[:])
            nc.sync.dma_start(out=pool_part[0:P, :], in_=pT[:, 0:G])
            nc.sync.dma_start(out=pool_part[P : 2 * P, :], in_=pT[:, G : 2 * G])
            nc.gpsimd.collective_compute(
                "AllReduce", OP.add,
                replica_groups=[list(range(NCORES))],
                ins=[pool_part[:, :]], outs=[pool_sum[:, :]],
            )
            ps = hpool.tile([P, 2 * G], F32, tag="ps_in")
            nc.sync.dma_start(out=ps[:, 0:G], in_=pool_sum[0:P, :])
            nc.sync.dma_start(out=ps[:, G : 2 * G], in_=pool_sum[P : 2 * P, :])
            pm = hpool.tile([P, 2 * G], F32, tag="pm")
            nc.vector.tensor_tensor(pm[:], ps[:], cntinv2_sb[:], OP.mult)
            gm, gu = selu_mu(hpool, pm[:], [P, 2 * G], BF16, "hd1")

            psum_fc1 = ps_h2.tile([P, G], F32, tag="main")
            for j in range(2):
                for pi, part in enumerate((gm, gu)):
                    nc.tensor.matmul(
                        psum_fc1[:], fc1_sb[:, j * d_fc : (j + 1) * d_fc],
                        part[:, j * G : (j + 1) * G],
                        start=(j == 0 and pi == 0), stop=(j == 1 and pi == 1),
                    )
            hm, hu = selu_mu(hpool, psum_fc1[:], [P, G], BF16, "hd2",
                             bias=fc1b_sb[:, 0:1], nbias=nfc1b_sb[:, 0:1])

            psum_fc2 = ps_tr.tile([n_cls, G], F32, tag="sm")
            nc.tensor.matmul(psum_fc2[:], fc2_sb[:], hm[:], start=True, stop=False)
            nc.tensor.matmul(psum_fc2[:], fc2_sb[:], hu[:], start=False, stop=True)
            lg2 = wpool.tile([n_cls, G], F32, tag="lg2")
            nc.scalar.activation(
                lg2[:], psum_fc2[:], AF.Identity, bias=fc2b_sb[0:n_cls, 0:1]
            )
            for j in range(-(-G // P)):
                gw = min(P, G - j * P)
                psT2 = ps_tr.tile([P, n_cls], F32, tag="sm")
                nc.tensor.transpose(
                    psT2[:gw, :], lg2[:, j * P : j * P + gw],
                    ident_sb[0:n_cls, 0:n_cls],
                )
                lgj = hpool.tile([P, n_cls], F32, tag="lgj")
                nc.scalar.copy(lgj[:gw, :], psT2[:gw, :])
                nm = hpool.tile([P, 1], F32, tag="nm")
                nc.vector.tensor_reduce(
                    nm[:gw, :], lgj[:gw, :], mybir.AxisListType.X, OP.max, negate=True
                )
                e4 = hpool.tile([P, n_cls], F32, tag="e4")
                nc.scalar.activation(e4[:gw, :], lgj[:gw, :], AF.Exp, bias=nm[:gw, 0:1])
                s4 = hpool.tile([P, 1], F32, tag="s4")
                nc.vector.tensor_reduce(s4[:gw, :], e4[:gw, :], mybir.AxisListType.X, OP.add)
                ls = hpool.tile([P, 1], F32, tag="ls")
                nc.scalar.activation(ls[:gw, :], s4[:gw, :], AF.Ln)
                q = hpool.tile([P, 1], F32, tag="q")
                nc.vector.tensor_tensor(q[:gw, :], nm[:gw, :], ls[:gw, :], OP.subtract)
                outj = hpool.tile([P, n_cls], F32, tag="outj")
                nc.vector.tensor_scalar(outj[:gw, :], lgj[:gw, :], q[:gw, 0:1], None, OP.add)
                nc.sync.dma_start(out=out_d[j * P : j * P + gw, :], in_=outj[:gw, :])

    nc.compile()
    return nc


_CACHE = {}


def run_gcn(inputs, n_nodes, n_graphs, d_in=14, d_hid=256, d_fc=128, n_cls=2,
            grp1=4, grp2=4, trace=False):
    split = LO_SHARDS * (n_nodes // NCORES)
    s, d, cut, nlo, nhi, cnt1 = edge_partition(inputs, n_nodes, split)
    l1 = Lay1(n_nodes, cnt1, grp1)
    l2 = Lay2(n_nodes, nlo, nhi, grp2)
    per_core, shared = host_prep(inputs, s, d, cut, l1, l2, n_nodes, n_graphs, split)
    key = (n_nodes, n_graphs, tuple(l1.T), tuple(l2.T_LO), tuple(l2.T_HI),
           grp1, grp2, shared["has_b1"], shared["has_b2"])
    if key not in _CACHE:
        _CACHE[key] = build_nc(l1, l2, n_nodes, n_graphs, d_hid, d_fc, n_cls,
                               shared["has_b1"], shared["has_b2"])
    nc = _CACHE[key]
    res = run_bass_kernel_spmd(nc, per_core, list(range(NCORES)), trace=trace)
    return res.results[0]["out"].astype(np.float32), res


def kernel(**inputs) -> np.ndarray:
    out, _ = run_gcn(
        inputs, n_nodes=50000, n_graphs=256,
        trace=bool(int(os.environ.get("GCN_TRACE", "0"))),
    )
    return out
